# revision 29
# baseline (speedup 1.0000x reference)
"""Trainium2 Bass kernel for AtlasMAGBlock (B=1, S=2048, D=1024, H=16, HD=64).

Sharding: 2 heads per core (tensor parallel) for qkv/omega/attention/o_proj;
FFN column-sharded 8 ways. One AllReduce (8.4MB) joins the attention residual
before the FFN. Host does layout transforms only (transposes / slicing /
constant tables); all FLOPs on x run on device.

Device layouts are feature-major ("transposed"): activations [feature, token]
so every matmul contracts over the partition axis. Per-token (free-axis) scale
rows are broadcast across partitions with K=1 matmuls against a ones column.
"""

import numpy as np
import ml_dtypes

import concourse.bass as bass
import concourse.tile as tile
from concourse import bacc, mybir
from concourse.bass_utils import run_bass_kernel_spmd

F32 = mybir.dt.float32
F32R = mybir.dt.float32r
BF16 = mybir.dt.bfloat16
AF = mybir.ActivationFunctionType
ALU = mybir.AluOpType

S, D, H, HD, GH = 2048, 1024, 16, 64, 64
DEBUG = False
FFN_H = 2730
CTX, LAM = 512, 0.999
NCORES = 8
HPC = H // NCORES          # heads per core = 2
FC = 342                   # FFN cols per core (8*342 = 2736, last 6 padded)
KT = D // 128              # 8 k-tiles over D
TB = S // 128              # 16 token blocks
NCH = S // 512             # 4 token chunks
F_SIZES = [128, 128, 86]   # FFN f-tile partition sizes (sum = 342)
F_OFFS = [0, 128, 256]


def _f32r(ap):
    return ap.bitcast(F32R)


def build_program():
    nc = bacc.Bacc("TRN2", target_bir_lowering=False, debug=False,
                   num_devices=NCORES)

    dt_in = {}

    def din(name, shape, dt=F32):
        h = nc.dram_tensor(name, list(shape), dt, kind="ExternalInput")
        dt_in[name] = h.ap()
        return dt_in[name]

    din("xT", (D, S), F32R)
    din("wqg", (D, 448), F32R)            # q0 q1 k0 k1 v0 v1 gamma_w1
    din("g2", (GH, 1), F32R)
    din("cosq", (HD, S))
    din("sinq", (HD, S))
    din("cosk", (HD, S))
    din("sink", (HD, S))
    din("mp", (HD, HPC * HD), BF16)  # m_persist[h].T per head
    din("wo0", (HD, D), F32R)
    din("wo1", (HD, D), F32R)
    din("w1", (D, FC), F32R)
    din("w3", (D, FC), F32R)
    din("w2b", (FC, D), BF16)
    din("aa", (1, S))                # lam^-i
    din("invb64", (64, S))           # lam^-t replicated over 64 partitions
    din("bb", (1, S))                # lam^t
    din("mdiag", (128, 128), F32R)         # tt >= ii
    din("mfar", (128, 128), F32R)          # tt < ii
    din("mones", (128, 128), F32R)
    din("amask4", (128, 4 * 512), BF16)
    din("ident64", (HD, HD), F32R)
    din("ones65", (65, 1), F32R)           # 1/64
    din("ones128c", (128, 1), F32R)        # 1/1024
    din("ones_r", (1, 128), F32R)          # 1.0 (K=1 broadcast matmuls)
    din("memgate64", (64, 1))
    din("eps11", (1, 1))
    din("vones", (128, TB * 65), BF16)

    yT = nc.dram_tensor("yT", [D, S], F32, kind="ExternalOutput").ap()
    if DEBUG:
        for nm, shp in [("d_qraw0", (64, S)), ("d_kh0", (64, S)),
                        ("d_grow", (1, S)), ("d_gacol", (128, TB)),
                        ("d_cscb", (64, S)), ("d_qtil0", (64, S)),
                        ("d_ao0", (64, S)), ("d_out1", (D, S)),
                        ("d_x2_0", (128, S)), ("d_r2b", (128, S)),
                        ("d_ktok0", (128, TB * 64)), ("d_vtok0", (128, TB * 65))]:
            dt_in[nm] = nc.dram_tensor(nm, list(shp),
                                       BF16 if "tok" in nm else F32,
                                       kind="ExternalOutput").ap()

    with tile.TileContext(nc) as tc:
        _emit(nc, tc, dt_in, yT)

    nc.compile()
    return nc


def _emit(nc, tc, I, yT):
    P_ctx = tc.tile_pool(name="persist", bufs=1)
    dram_ctx = tc.tile_pool(name="dram", bufs=1, space="DRAM")
    with P_ctx as P, dram_ctx as dramp, nc.allow_low_precision('deliberate f32r/bf16 design'):
        def load(pool, name, shape=None, dt=F32, src=None, tag=None):
            src = src if src is not None else I[name]
            shape = list(shape) if shape is not None else list(src.shape)
            t = pool.tile(shape, dt, tag=(tag or name), name=(tag or name))
            nc.sync.dma_start(t[:], src)
            return t

        # ---- small persistent consts ----
        g2_sb = load(P, "g2", dt=F32R)
        mp_sb = load(P, "mp", dt=BF16)
        aa_sb = load(P, "aa")
        bb_sb = load(P, "bb")
        mdiag_sb = load(P, "mdiag", dt=F32R)
        mfar_sb = load(P, "mfar", dt=F32R)
        mones_sb = load(P, "mones", dt=F32R)
        amask_sb = load(P, "amask4", dt=BF16)
        id64_sb = load(P, "ident64", dt=F32R)
        ones65_sb = load(P, "ones65", dt=F32R)
        ones128c_sb = load(P, "ones128c", dt=F32R)
        ones_r_sb = load(P, "ones_r", dt=F32R)
        mg_sb = load(P, "memgate64")
        eps_sb = load(P, "eps11")
        invb64_sb = load(P, "invb64")

        # memory gate scalars, broadcast over 64 partitions
        g_b = P.tile([64, 1], F32, tag="g_b")
        gm1_b = P.tile([64, 1], F32, tag="gm1_b")
        nc.scalar.activation(g_b[:], mg_sb[:], AF.Sigmoid)
        nc.scalar.activation(gm1_b[:], mg_sb[:], AF.Sigmoid, scale=-1.0)

        # mid-lived activation tiles (through attention)
        mid_ctx = tc.tile_pool(name="mid", bufs=1)
        with mid_ctx as M:
            vT = [M.tile([64, S], F32R, tag=f"vT{h}", name=f"vT{h}") for h in range(HPC)]
            qh_t = [M.tile([64, S], F32R, tag=f"qh{h}", name=f"qh{h}") for h in range(HPC)]
            kh_t = [M.tile([64, S], F32R, tag=f"kh{h}", name=f"kh{h}") for h in range(HPC)]
            ga_col = M.tile([128, TB], F32R, tag="ga_col")
            csc_b = M.tile([64, S], F32, tag="csc_b")

            # ============ phase A: qkv + gamma-hidden matmuls ============
            with tc.tile_pool(name="qkvout", bufs=1) as QO:
                qraw = [QO.tile([64, S], F32, tag=f"qraw{h}", name=f"qraw{h}")
                        for h in range(HPC)]
                kraw = [QO.tile([64, S], F32, tag=f"kraw{h}", name=f"kraw{h}")
                        for h in range(HPC)]
                guT = QO.tile([64, S], F32, tag="guT")
                with tc.tile_pool(name="xp", bufs=2) as xp, \
                     tc.tile_pool(name="wqgp", bufs=1) as wp, \
                     tc.tile_pool(name="ps_qkv", bufs=4, space="PSUM") as pq, \
                     tc.tile_pool(name="ps_qx", bufs=2, space="PSUM") as pq2:
                    wqg_sb = [load(wp, "wqg", (128, 448), dt=F32R,
                                   src=I["wqg"][k * 128:(k + 1) * 128, :],
                                   tag=f"wqg{k}") for k in range(KT)]
                    m_outs = [(qraw[0], 0), (qraw[1], 64), (kraw[0], 128),
                              (kraw[1], 192), (vT[0], 256), (vT[1], 320),
                              (guT, 384)]
                    for c in range(NCH):
                        csl = slice(c * 512, (c + 1) * 512)
                        x_sb = []
                        for k in range(KT):
                            t = xp.tile([128, 512], F32R, tag=f"x{k}", name=f"x{k}_{c}")
                            nc.sync.dma_start(
                                t[:], I["xT"][k * 128:(k + 1) * 128, csl])
                            x_sb.append(t)
                        # r1 = rsqrt(mean_d x^2 + eps) for this chunk
                        ps_x = pq2.tile([1, 512], F32, tag="ps_x")
                        for k in range(KT):
                            sqx = xp.tile([128, 512], F32R, tag="sqx")
                            nc.scalar.activation(sqx[:], x_sb[k].bitcast(F32)[:],
                                                 AF.Square)
                            nc.tensor.matmul(ps_x[:], ones128c_sb[:], sqx[:],
                                             start=(k == 0), stop=(k == KT - 1))
                        m1x = xp.tile([1, 512], F32, tag="m1x")
                        nc.scalar.activation(m1x[:], ps_x[:], AF.Identity,
                                             bias=eps_sb[:])
                        rc1 = xp.tile([1, 512], F32, tag="rc1")
                        nc.vector.reciprocal(rc1[:], m1x[:])
                        r1row = xp.tile([1, 512], F32R, tag="r1row")
                        nc.scalar.activation(r1row[:], rc1[:], AF.Sqrt)
                        pb1 = pq2.tile([64, 512], F32, tag="pb1")
                        nc.tensor.matmul(pb1[:], ones_r_sb[:, 0:64], r1row[:],
                                         start=True, stop=True)
                        r1b = xp.tile([64, 512], F32, tag="r1b")
                        nc.any.tensor_copy(r1b[:], pb1[:])
                        for (dst, moff) in m_outs:
                            ps = pq.tile([64, 512], F32, tag="ps")
                            for k in range(KT):
                                nc.tensor.matmul(
                                    ps[:], _f32r(wqg_sb[k][:, moff:moff + 64]),
                                    _f32r(x_sb[k][:]),
                                    start=(k == 0), stop=(k == KT - 1))
                            nc.vector.tensor_mul(dst[:, csl], ps[:], r1b[:])

                if DEBUG:
                    nc.sync.dma_start(I["d_qraw0"], qraw[0].bitcast(F32)[:])
                # ============ gamma gate rows ============
                with tc.tile_pool(name="gp", bufs=2) as gp, \
                     tc.tile_pool(name="ps_g", bufs=1, space="PSUM") as pg:
                    gsil = gp.tile([64, S], F32R, tag="gsil")
                    nc.scalar.activation(gsil[:], guT[:], AF.Silu)
                    grow = gp.tile([1, S], F32, tag="grow")
                    for c in range(NCH):
                        csl = slice(c * 512, (c + 1) * 512)
                        psg = pg.tile([1, 512], F32, tag="psg")
                        nc.tensor.matmul(psg[:], _f32r(g2_sb[:]),
                                         _f32r(gsil[:, csl]), start=True, stop=True)
                        nc.scalar.activation(grow[:, csl], psg[:], AF.Sigmoid)
                    if DEBUG:
                        nc.sync.dma_start(I["d_grow"], grow[:])
                    ga_row = gp.tile([1, S], F32R, tag="ga_row")
                    nc.vector.tensor_mul(ga_row[:], grow[:], aa_sb[:])
                    ga_dram = dramp.tile([1, S], F32R, tag="ga_dram")
                    nc.sync.dma_start(ga_dram[:], ga_row[:])
                    with nc.allow_non_contiguous_dma("one-time gamma transpose"):
                        nc.sync.dma_start(
                            ga_col[:], ga_dram.rearrange("a (j p) -> (a p) j", p=128))

                    # norm row: csc[t] = lam^t / (1 + lam^t * sum_i ga_i mask)
                    with tc.tile_pool(name="ps_bc", bufs=2, space="PSUM") as pbc:
                        for cc in range(NCH):
                            csl = slice(cc * 512, (cc + 1) * 512)
                            ps_ns = pg.tile([1, 512], F32, tag="ps_ns")
                            for tt in range(4):
                                tb = cc * 4 + tt
                                ibs = list(range(max(0, tb - 4), tb + 1))
                                for j, ib in enumerate(ibs):
                                    msk = mdiag_sb if ib == tb else (
                                        mfar_sb if ib == tb - 4 else mones_sb)
                                    nc.tensor.matmul(
                                        ps_ns[:, tt * 128:(tt + 1) * 128],
                                        ga_col[:, ib:ib + 1], msk[:],
                                        start=(j == 0), stop=(j == len(ibs) - 1))
                            nrm = gp.tile([1, 512], F32, tag="nrm")
                            nc.vector.tensor_mul(nrm[:], ps_ns[:], bb_sb[:, csl])
                            nc.vector.tensor_scalar_add(nrm[:], nrm[:], 1.0)
                            rcpn = gp.tile([1, 512], F32, tag="rcpn")
                            nc.vector.reciprocal(rcpn[:], nrm[:])
                            csc_row = gp.tile([1, 512], F32R, tag="csc_row")
                            nc.vector.tensor_mul(csc_row[:], rcpn[:], bb_sb[:, csl])
                            pb = pbc.tile([64, 512], F32, tag="pb")
                            nc.tensor.matmul(pb[:], _f32r(ones_r_sb[:, 0:64]),
                                             _f32r(csc_row[:]), start=True, stop=True)
                            nc.any.tensor_copy(csc_b[:, csl], pb[:])

                # ============ phase B: qk rms-norm + rope (both heads) ============
                with tc.tile_pool(name="ropes", bufs=1) as RT, \
                     tc.tile_pool(name="ropew", bufs=1) as rp, \
                     tc.tile_pool(name="ps_r", bufs=2, space="PSUM") as pr:
                    cosq_sb = load(RT, "cosq"); sinq_sb = load(RT, "sinq")
                    cosk_sb = load(RT, "cosk"); sink_sb = load(RT, "sink")
                    for h in range(HPC):
                        for nm, src, cs, sn, dst in (
                                ("q", qraw[h][:], cosq_sb, sinq_sb, qh_t[h]),
                                ("k", kraw[h][:], cosk_sb, sink_sb, kh_t[h])):
                            # rms scale row: rsqrt(mean + eps)
                            sq = rp.tile([65, S], F32R, tag="sq")
                            nc.vector.tensor_mul(sq[0:64, :], src, src)
                            nc.gpsimd.memset(sq.bitcast(F32)[64:65, :], 6.4e-5)
                            rro = rp.tile([1, S], F32R, tag="rro")
                            for c in range(NCH):
                                csl = slice(c * 512, (c + 1) * 512)
                                ps = pr.tile([1, 512], F32, tag="ps")
                                nc.tensor.matmul(ps[:], _f32r(ones65_sb[:]),
                                                 _f32r(sq[:, csl]), start=True, stop=True)
                                rc = rp.tile([1, 512], F32, tag="rc")
                                nc.vector.reciprocal(rc[:], ps[:])
                                nc.scalar.activation(rro[:, csl], rc[:], AF.Sqrt)
                            # rope
                            sh = rp.tile([64, S], F32, tag="sh")
                            nc.sync.dma_start(sh[0:32, :], src[32:64, :])
                            nc.sync.dma_start(sh[32:64, :], src[0:32, :])
                            t1 = rp.tile([64, S], F32, tag="t1")
                            nc.vector.tensor_mul(t1[:], src, cs[:])
                            nc.gpsimd.tensor_mul(sh[:], sh[:], sn[:])
                            nc.vector.tensor_add(t1[:], t1[:], sh[:])
                            for c in range(NCH):
                                csl = slice(c * 512, (c + 1) * 512)
                                pb = pr.tile([64, 512], F32, tag="pb")
                                nc.tensor.matmul(pb[:], _f32r(ones_r_sb[:, 0:64]),
                                                 _f32r(rro[:, csl]), start=True, stop=True)
                                nc.vector.tensor_mul(dst[:, csl], t1[:, csl], pb[:])

            if DEBUG:
                nc.sync.dma_start(I["d_gacol"], ga_col.bitcast(F32)[:])
                nc.sync.dma_start(I["d_cscb"], csc_b[:])
                nc.sync.dma_start(I["d_kh0"], kh_t[0].bitcast(F32)[:])
            # ============ phase C: per-head omega + attention ============
            aop_cm = tc.tile_pool(name="aop", bufs=1)
            aop = aop_cm.__enter__()
            ao_t = [aop.tile([64, S], F32R, tag=f"ao{h}", name=f"ao{h}")
                    for h in range(HPC)]
            for h in range(HPC):
                qh, kh, vraw = qh_t[h], kh_t[h], vT[h]
                with tc.tile_pool(name=f"head{h}", bufs=1) as hp:
                    qhb = hp.tile([64, S], BF16, tag="qhb")
                    nc.vector.tensor_copy(qhb[:], qh.bitcast(F32)[:])
                    qomb = hp.tile([64, S], BF16, tag="qomb")
                    nc.vector.tensor_mul(qomb[:], qh.bitcast(F32)[:], invb64_sb[:])
                    khb = hp.tile([64, S], BF16, tag="khb")
                    nc.vector.tensor_copy(khb[:], kh.bitcast(F32)[:])

                    ktok = hp.tile([128, TB * 64], BF16, tag="ktok")
                    vtok = hp.tile([128, TB * 65], BF16, tag="vtok")
                    nc.sync.dma_start(vtok[:], I["vones"])
                    with tc.tile_pool(name=f"tp{h}", bufs=4, space="PSUM") as tp:
                        for j in range(TB):
                            pt = tp.tile([128, 64], F32R, tag="pt")
                            nc.tensor.transpose(pt[:], kh[:, j * 128:(j + 1) * 128],
                                                id64_sb[:])
                            nc.any.tensor_copy(ktok[:, j * 64:(j + 1) * 64], pt.bitcast(F32)[:])
                            pt2 = tp.tile([128, 64], F32R, tag="pt2")
                            nc.tensor.transpose(pt2[:], vraw[:, j * 128:(j + 1) * 128],
                                                id64_sb[:])
                            nc.any.tensor_copy(vtok[:, j * 65:j * 65 + 64], pt2.bitcast(F32)[:])

                    # ---- omega -> qtil ----
                    qtil = hp.tile([64, S], F32R, tag="qtil")
                    with tc.tile_pool(name=f"om{h}", bufs=3) as om, \
                         tc.tile_pool(name=f"ps_s{h}", bufs=2, space="PSUM") as pss, \
                         tc.tile_pool(name=f"ps_c{h}", bufs=2, space="PSUM") as psc:
                        for ch in range(NCH):
                            ps_c4 = psc.tile([64, 512], F32, tag="ps_c4")
                            for tt in range(4):
                                tb = ch * 4 + tt
                                ibs = list(range(max(0, tb - 4), tb + 1))
                                ps_s = pss.tile([128, 640], F32, tag="ps_s")
                                ws = om.tile([128, 640], BF16, tag="ws")
                                for j, ib in enumerate(ibs):
                                    jsl = (slice(None), slice(j * 128, (j + 1) * 128))
                                    nc.tensor.matmul(ps_s[jsl],
                                                     khb[:, ib * 128:(ib + 1) * 128],
                                                     qhb[:, tb * 128:(tb + 1) * 128],
                                                     start=True, stop=True)
                                    gcol = ga_col.bitcast(F32)[:, ib:ib + 1]
                                    if ib == tb:
                                        nc.vector.scalar_tensor_tensor(
                                            ws[jsl], ps_s[jsl], gcol, mdiag_sb.bitcast(F32)[:],
                                            op0=ALU.mult, op1=ALU.mult)
                                    elif ib == tb - 4:
                                        nc.vector.scalar_tensor_tensor(
                                            ws[jsl], ps_s[jsl], gcol, mfar_sb.bitcast(F32)[:],
                                            op0=ALU.mult, op1=ALU.mult)
                                    else:
                                        nc.vector.tensor_scalar_mul(
                                            ws[jsl], ps_s[jsl], gcol)
                                osl = (slice(None), slice(tt * 128, (tt + 1) * 128))
                                nc.tensor.matmul(ps_c4[osl],
                                                 mp_sb[:, h * 64:(h + 1) * 64],
                                                 qomb[:, tb * 128:(tb + 1) * 128],
                                                 start=True, stop=False)
                                for j, ib in enumerate(ibs):
                                    nc.tensor.matmul(ps_c4[osl],
                                                     ktok[:, ib * 64:(ib + 1) * 64],
                                                     ws[:, j * 128:(j + 1) * 128],
                                                     start=False,
                                                     stop=(j == len(ibs) - 1))
                            csl = slice(ch * 512, (ch + 1) * 512)
                            tq = om.tile([64, 512], F32, tag="tq")
                            nc.vector.scalar_tensor_tensor(
                                tq[:], ps_c4[:], g_b[:], csc_b[:, csl],
                                op0=ALU.mult, op1=ALU.mult)
                            nc.vector.scalar_tensor_tensor(
                                qtil[:, csl], qh.bitcast(F32)[:, csl], gm1_b[:], tq[:],
                                op0=ALU.mult, op1=ALU.add)

                    if DEBUG and h == 0:
                        nc.sync.dma_start(I["d_qtil0"], qtil.bitcast(F32)[:])
                        nc.sync.dma_start(I["d_ktok0"], ktok[:])
                        nc.sync.dma_start(I["d_vtok0"], vtok[:])
                    # ---- causal attention ----
                    avsp_cm = tc.tile_pool(name=f"avs{h}", bufs=1)
                    avsp = avsp_cm.__enter__()
                    with tc.tile_pool(name=f"at{h}", bufs=3) as at, \
                         tc.tile_pool(name=f"ps_a{h}", bufs=2, space="PSUM") as psa, \
                         tc.tile_pool(name=f"ps_av{h}", bufs=1, space="PSUM") as psav:
                        av_ps = [psav.tile([65, 512], F32, tag=f"av{c}",
                                           name=f"av{h}_{c}") for c in range(NCH)]
                        avs = [avsp.tile([65, 512], F32, tag=f"avs{c}",
                                         name=f"avs{h}_{c}") for c in range(NCH)]
                        for half in range(2):
                            for ib in range(8 * half + 8):
                                ps_a = psa.tile([128, 1024], F32, tag="ps_a")
                                ab = at.tile([128, 1024], BF16, tag="ab")
                                vs = None
                                for sc in range(2):
                                    c = half * 2 + sc
                                    if (c + 1) * 512 <= ib * 128:
                                        continue
                                    if vs is None:
                                        vs = sc
                                    nc.tensor.matmul(
                                        ps_a[:, sc * 512:(sc + 1) * 512],
                                        _f32r(kh[:, ib * 128:(ib + 1) * 128]),
                                        _f32r(qtil[:, (c * 512):(c + 1) * 512]),
                                        start=True, stop=True)
                                esl = (slice(None), slice(vs * 512, 1024))
                                nc.scalar.activation(ab[esl], ps_a[esl], AF.Exp,
                                                     scale=HD ** -0.5)
                                cd = (ib * 128) // 512
                                r = ib - 4 * cd
                                scd = cd - 2 * half
                                if scd >= 0:
                                    dsl = (slice(None),
                                           slice(scd * 512, (scd + 1) * 512))
                                    nc.vector.tensor_mul(
                                        ab[dsl], ab[dsl],
                                        amask_sb[:, r * 512:(r + 1) * 512])
                                for sc in range(2):
                                    c = half * 2 + sc
                                    if (c + 1) * 512 <= ib * 128:
                                        continue
                                    nc.tensor.matmul(
                                        av_ps[c][:], vtok[:, ib * 65:(ib + 1) * 65],
                                        ab[:, sc * 512:(sc + 1) * 512],
                                        start=(ib == 0), stop=(ib == 4 * c + 3))
                            for sc in range(2):
                                c = half * 2 + sc
                                nc.any.tensor_copy(avs[c][:], av_ps[c][:])
                    # normalize: aoT = avs[:64] * (1/avs[64]) broadcast
                    with tc.tile_pool(name=f"nz{h}", bufs=2) as nz, \
                         tc.tile_pool(name=f"ps_rc{h}", bufs=2, space="PSUM") as prc:
                        for c in range(NCH):
                            csl = slice(c * 512, (c + 1) * 512)
                            rcp = nz.tile([1, 512], F32R, tag="rcp")
                            nc.vector.reciprocal(rcp[:], avs[c][64:65, :])
                            pb = prc.tile([64, 512], F32, tag="pb")
                            nc.tensor.matmul(pb[:], _f32r(ones_r_sb[:, 0:64]),
                                             _f32r(rcp[:]), start=True, stop=True)
                            nc.vector.tensor_mul(
                                ao_t[h][:, csl], avs[c][0:64, :], pb[:])
                    avsp_cm.__exit__(None, None, None)

            if DEBUG:
                nc.sync.dma_start(I["d_ao0"], ao_t[0].bitcast(F32)[:])
            # ============ phase D: o_proj partial -> DRAM -> AllReduce ============
            out1 = dramp.tile([D, S], F32, tag="out1")
            arout = dramp.tile([D, S], F32, tag="arout", addr_space="Shared")
            with tc.tile_pool(name="wop", bufs=1) as wop, \
                 tc.tile_pool(name="ps_o", bufs=4, space="PSUM") as pso, \
                 tc.tile_pool(name="ob", bufs=4) as ob:
                wo_sb = [load(wop, "wo0", dt=F32R), load(wop, "wo1", dt=F32R)]
                for m in range(KT):
                    for c in range(NCH):
                        csl = slice(c * 512, (c + 1) * 512)
                        ps = pso.tile([128, 512], F32, tag="ps")
                        for h in range(HPC):
                            nc.tensor.matmul(
                                ps[:], _f32r(wo_sb[h][:, m * 128:(m + 1) * 128]),
                                _f32r(ao_t[h][:, csl]),
                                start=(h == 0), stop=(h == HPC - 1))
                        o_sb = ob.tile([128, 512], F32, tag="o_sb")
                        nc.any.tensor_copy(o_sb[:], ps[:])
                        nc.sync.dma_start(out1[m * 128:(m + 1) * 128, csl], o_sb[:])
            nc.gpsimd.collective_compute(
                "AllReduce", ALU.add, replica_groups=[list(range(NCORES))],
                ins=[out1[:]], outs=[arout[:]])
            if DEBUG:
                nc.sync.dma_start(I["d_out1"], out1[:])
            aop_cm.__exit__(None, None, None)

        # ============ phase E: x2 = x + AR, rms2 stats ============
        with tc.tile_pool(name="x2p", bufs=1) as x2p, \
             tc.tile_pool(name="w123", bufs=1) as wfp:
            w1_sb = [load(wfp, "w1", (128, FC), dt=F32R, src=I["w1"][k * 128:(k + 1) * 128, :],
                          tag=f"w1{k}") for k in range(KT)]
            w3_sb = [load(wfp, "w3", (128, FC), dt=F32R, src=I["w3"][k * 128:(k + 1) * 128, :],
                          tag=f"w3{k}") for k in range(KT)]
            w2_sb = [load(wfp, "w2b", (F_SIZES[f], D), dt=BF16,
                          src=I["w2b"][F_OFFS[f]:F_OFFS[f] + F_SIZES[f], :],
                          tag=f"w2{f}") for f in range(3)]
            x2_sb = [x2p.tile([128, S], F32R, tag=f"x2_{d}", name=f"x2_{d}")
                     for d in range(KT)]
            r2b = x2p.tile([128, S], F32, tag="r2b")
            with tc.tile_pool(name="sqp", bufs=2) as sqp, \
                 tc.tile_pool(name="ps_s2", bufs=1, space="PSUM") as ps2p:
                ps_sq = [ps2p.tile([1, 512], F32, tag=f"ps_sq{c}", name=f"ps_sq{c}")
                         for c in range(NCH)]
                for d in range(KT):
                    tx = sqp.tile([128, S], F32R, tag="tx")
                    nc.sync.dma_start(tx[:], I["xT"][d * 128:(d + 1) * 128, :])
                    ta = sqp.tile([128, S], F32, tag="ta")
                    nc.sync.dma_start(ta[:], arout[d * 128:(d + 1) * 128, :])
                    nc.gpsimd.tensor_add(x2_sb[d][:], tx.bitcast(F32)[:], ta[:])
                    sq = sqp.tile([128, S], F32R, tag="sq")
                    nc.scalar.activation(sq[:], x2_sb[d].bitcast(F32)[:], AF.Square)
                    for c in range(NCH):
                        csl = slice(c * 512, (c + 1) * 512)
                        nc.tensor.matmul(ps_sq[c][:], _f32r(ones128c_sb[:]),
                                         _f32r(sq[:, csl]),
                                         start=(d == 0), stop=(d == KT - 1))
                r2row = sqp.tile([1, S], F32R, tag="r2row")
                for c in range(NCH):
                    m2 = sqp.tile([1, 512], F32, tag="m2")
                    nc.scalar.activation(m2[:], ps_sq[c][:], AF.Identity, bias=eps_sb[:])
                    rc2 = sqp.tile([1, 512], F32, tag="rc2")
                    nc.vector.reciprocal(rc2[:], m2[:])
                    nc.scalar.activation(r2row[:, c * 512:(c + 1) * 512], rc2[:], AF.Sqrt)
                with tc.tile_pool(name="ps_b2", bufs=2, space="PSUM") as pb2:
                    for c in range(NCH):
                        csl = slice(c * 512, (c + 1) * 512)
                        pb = pb2.tile([128, 512], F32, tag="pb")
                        nc.tensor.matmul(pb[:], _f32r(ones_r_sb[:]),
                                         _f32r(r2row[:, csl]), start=True, stop=True)
                        nc.any.tensor_copy(r2b[:, csl], pb[:])

            if DEBUG:
                nc.sync.dma_start(I["d_x2_0"], x2_sb[0].bitcast(F32)[:])
                nc.sync.dma_start(I["d_r2b"], r2b[:])
            # ============ phase F: FFN (column shard) + residual/8 ============
            with tc.tile_pool(name="ffn", bufs=3) as fp, \
                 tc.tile_pool(name="gpool", bufs=1) as gpl, \
                 tc.tile_pool(name="ps_f", bufs=2, space="PSUM") as pf:
                g_sb = [gpl.tile([F_SIZES[f], S], BF16, tag=f"g{f}", name=f"g{f}")
                        for f in range(3)]
                for f in range(3):
                    fo, fs = F_OFFS[f], F_SIZES[f]
                    for c in range(NCH):
                        csl = slice(c * 512, (c + 1) * 512)
                        ps1 = pf.tile([fs, 512], F32, tag="ps1")
                        ps3 = pf.tile([fs, 512], F32, tag="ps3")
                        for k in range(KT):
                            nc.tensor.matmul(ps1[:], _f32r(w1_sb[k][:, fo:fo + fs]),
                                             _f32r(x2_sb[k][:, csl]),
                                             start=(k == 0), stop=(k == KT - 1))
                        for k in range(KT):
                            nc.tensor.matmul(ps3[:], _f32r(w3_sb[k][:, fo:fo + fs]),
                                             _f32r(x2_sb[k][:, csl]),
                                             start=(k == 0), stop=(k == KT - 1))
                        u1 = fp.tile([fs, 512], F32, tag="u1")
                        nc.vector.tensor_mul(u1[:], ps1[:], r2b[0:fs, csl])
                        su = fp.tile([fs, 512], F32, tag="su")
                        nc.scalar.activation(su[:], u1[:], AF.Silu)
                        u3 = fp.tile([fs, 512], F32, tag="u3")
                        nc.vector.tensor_mul(u3[:], ps3[:], r2b[0:fs, csl])
                        nc.vector.tensor_mul(g_sb[f][:, csl], su[:], u3[:])
                for m in range(KT):
                    for c in range(NCH):
                        csl = slice(c * 512, (c + 1) * 512)
                        psy = pf.tile([128, 512], F32, tag="psy")
                        for f in range(3):
                            nc.tensor.matmul(psy[:], w2_sb[f][:, m * 128:(m + 1) * 128],
                                             g_sb[f][:, csl],
                                             start=(f == 0), stop=(f == 2))
                        ysb = fp.tile([128, 512], F32, tag="ysb")
                        eng = nc.vector
                        eng.scalar_tensor_tensor(ysb[:], x2_sb[m].bitcast(F32)[:, csl],
                                                 1.0 / NCORES, psy[:],
                                                 op0=ALU.mult, op1=ALU.add)
                        nc.sync.dma_start(yT[m * 128:(m + 1) * 128, csl], ysb[:])


_NC_CACHE = None


def _get_program():
    global _NC_CACHE
    if _NC_CACHE is None:
        _NC_CACHE = build_program()
    return _NC_CACHE


def _host_prepare(inputs):
    x = np.asarray(inputs["x"], np.float32)
    n1 = np.asarray(inputs["norm1_w"], np.float32)
    n2 = np.asarray(inputs["norm2_w"], np.float32)
    w_qkv = np.asarray(inputs["w_qkv"], np.float32)
    qn = np.asarray(inputs["q_norm_w"], np.float32)
    kn = np.asarray(inputs["k_norm_w"], np.float32)
    gw1 = np.asarray(inputs["gamma_w1"], np.float32)
    gw2 = np.asarray(inputs["gamma_w2"], np.float32)
    mper = np.asarray(inputs["m_persist"], np.float32)
    mg = np.asarray(inputs["memory_gate"], np.float32)
    w_o = np.asarray(inputs["w_o"], np.float32)
    fw1 = np.asarray(inputs["ffn_w1"], np.float32)
    fw3 = np.asarray(inputs["ffn_w3"], np.float32)
    fw2 = np.asarray(inputs["ffn_w2"], np.float32)

    xT = np.ascontiguousarray(x[0].T)

    t = np.arange(S, dtype=np.float32)
    inv = (1.0 / (10000.0 ** (np.arange(0, HD, 2, dtype=np.float32) / HD)))
    fr = np.outer(inv, t)                       # [32, S]
    c32, s32 = np.cos(fr), np.sin(fr)
    cos64 = np.concatenate([c32, c32], 0)       # [64, S]
    sin64 = np.concatenate([s32, s32], 0)
    sign = np.where(np.arange(HD) < 32, -1.0, 1.0).astype(np.float32)[:, None]

    def rope_tabs(w):
        cosw = cos64 * w[:, None]
        wsh = np.concatenate([w[32:], w[:32]])
        sinw = sin64 * sign * wsh[:, None]
        return (np.ascontiguousarray(cosw, dtype=np.float32),
                np.ascontiguousarray(sinw, dtype=np.float32))

    cosq, sinq = rope_tabs(qn)
    cosk, sink = rope_tabs(kn)

    i_idx = np.arange(S, dtype=np.float64)
    aa = (LAM ** (-i_idx)).astype(np.float32)[None, :]
    bb = (LAM ** i_idx).astype(np.float32)[None, :]
    invb64 = np.broadcast_to(aa, (64, S)).astype(np.float32)

    ii = np.arange(128)[:, None]
    u = np.arange(512)[None, :]
    mdiag = (np.arange(128)[None, :] >= ii).astype(np.float32)
    mfar = (np.arange(128)[None, :] < ii).astype(np.float32)
    mones = np.ones((128, 128), np.float32)
    amask4 = np.concatenate(
        [(u >= r * 128 + ii).astype(np.float32) for r in range(4)], axis=1
    ).astype(ml_dtypes.bfloat16)

    fw1p = np.zeros((D, NCORES * FC), np.float32)
    fw1p[:, :FFN_H] = fw1 * n2[:, None]
    fw3p = np.zeros((D, NCORES * FC), np.float32)
    fw3p[:, :FFN_H] = fw3 * n2[:, None]
    fw2p = np.zeros((NCORES * FC, D), np.float32)
    fw2p[:FFN_H, :] = fw2

    vones = np.zeros((128, TB * 65), ml_dtypes.bfloat16)
    for j in range(TB):
        vones[:, j * 65 + 64] = 1.0

    in_maps = []
    for c in range(NCORES):
        h0, h1 = HPC * c, HPC * c + 1
        cols = []
        for base in (0, D, 2 * D):
            for hh in (h0, h1):
                cols.append(w_qkv[:, base + hh * HD: base + (hh + 1) * HD])
        cols.append(gw1)
        wqg = np.concatenate(cols, axis=1) * n1[:, None]
        mp = np.concatenate([mper[h0].T, mper[h1].T], axis=1).astype(ml_dtypes.bfloat16)
        in_maps.append({
            "xT": xT, "wqg": np.ascontiguousarray(wqg),
            "g2": gw2,
            "cosq": cosq, "sinq": sinq, "cosk": cosk, "sink": sink,
            "mp": np.ascontiguousarray(mp),
            "wo0": np.ascontiguousarray(w_o[c * 128:c * 128 + 64, :]),
            "wo1": np.ascontiguousarray(w_o[c * 128 + 64:(c + 1) * 128, :]),
            "w1": np.ascontiguousarray(fw1p[:, c * FC:(c + 1) * FC]),
            "w3": np.ascontiguousarray(fw3p[:, c * FC:(c + 1) * FC]),
            "w2b": np.ascontiguousarray(
                fw2p[c * FC:(c + 1) * FC, :].astype(ml_dtypes.bfloat16)),
            "aa": aa, "invb64": invb64, "bb": bb,
            "mdiag": mdiag, "mfar": mfar, "mones": mones,
            "amask4": np.ascontiguousarray(amask4),
            "ident64": np.eye(HD, dtype=np.float32),
            "ones65": np.full((65, 1), 1.0 / HD, np.float32),
            "ones128c": np.full((128, 1), 1.0 / D, np.float32),
            "ones_r": np.ones((1, 128), np.float32),
            "memgate64": np.full((64, 1), float(mg.reshape(-1)[0]), np.float32),
            "eps11": np.full((1, 1), 1e-6, np.float32),
            "vones": vones,
        })
    return in_maps


def kernel(**inputs):
    nc = _get_program()
    in_maps = _host_prepare(inputs)
    res = run_bass_kernel_spmd(nc, in_maps, list(range(NCORES)))
    acc = None
    for c in range(NCORES):
        yt = np.asarray(res.results[c]["yT"], np.float32)
        acc = yt.copy() if acc is None else acc + yt
    return np.ascontiguousarray(acc.T)[None].astype(np.float32)


# revision 34
# speedup vs baseline: 1.5054x; 1.5054x over previous
"""Trainium2 Bass kernel for AtlasMAGBlock (B=1, S=2048, D=1024, H=16, HD=64).

Sharding: 2 heads per core (tensor parallel) for qkv/omega/attention/o_proj;
FFN column-sharded 8 ways. One AllReduce (8.4MB) joins the attention residual
before the FFN. Host does layout transforms only (transposes / slicing /
constant tables); all FLOPs on x run on device.

Device layouts are feature-major ("transposed"): activations [feature, token]
so every matmul contracts over the partition axis. Per-token (free-axis) scale
rows are broadcast across partitions with K=1 matmuls against a ones column.
"""

import numpy as np
import ml_dtypes

import concourse.bass as bass
import concourse.tile as tile
from concourse import bacc, mybir
from concourse.bass_utils import run_bass_kernel_spmd

F32 = mybir.dt.float32
F32R = mybir.dt.float32r
BF16 = mybir.dt.bfloat16
AF = mybir.ActivationFunctionType
ALU = mybir.AluOpType

S, D, H, HD, GH = 2048, 1024, 16, 64, 64
DEBUG = False
FFN_H = 2730
CTX, LAM = 512, 0.999
NCORES = 8
HPC = H // NCORES          # heads per core = 2
FC = 342                   # FFN cols per core (8*342 = 2736, last 6 padded)
KT = D // 128              # 8 k-tiles over D
TB = S // 128              # 16 token blocks
NCH = S // 512             # 4 token chunks
F_SIZES = [128, 128, 86]   # FFN f-tile partition sizes (sum = 342)
F_OFFS = [0, 128, 256]


def _f32r(ap):
    return ap.bitcast(F32R)


def build_program():
    nc = bacc.Bacc("TRN2", target_bir_lowering=False, debug=False,
                   num_devices=NCORES)

    dt_in = {}

    def din(name, shape, dt=F32):
        h = nc.dram_tensor(name, list(shape), dt, kind="ExternalInput")
        dt_in[name] = h.ap()
        return dt_in[name]

    din("xT", (D, S), F32R)
    din("wqg", (D, 448), F32R)            # q0 q1 k0 k1 v0 v1 gamma_w1
    din("g2", (GH, 1), F32R)
    din("cosq", (HD, S))
    din("sinq", (HD, S))
    din("cosk", (HD, S))
    din("sink", (HD, S))
    din("mp", (HD, HPC * HD), BF16)  # m_persist[h].T per head
    din("wob", (D, D), BF16)
    din("w1", (D, FC), F32R)
    din("w3", (D, FC), F32R)
    din("w2b", (FC, D), BF16)
    din("aa", (1, S))                # lam^-i
    din("invb64", (64, S))           # lam^-t replicated over 64 partitions
    din("bb", (1, S))                # lam^t
    din("mdiag", (128, 128), F32R)         # tt >= ii
    din("mfar", (128, 128), F32R)          # tt < ii
    din("mones", (128, 128), F32R)
    din("amask4", (128, 4 * 512), BF16)
    din("ident64", (HD, HD), F32R)
    din("ones65", (65, 1), F32R)           # 1/64
    din("ones128c", (128, 1), F32R)        # 1/1024
    din("ones_r", (1, 128), F32R)          # 1.0 (K=1 broadcast matmuls)
    din("memgate64", (64, 1))
    din("eps11", (1, 1))
    din("vones", (128, TB * 65), BF16)

    yT = nc.dram_tensor("yT", [D, S], F32, kind="ExternalOutput").ap()
    if DEBUG:
        for nm, shp in [("d_qraw0", (64, S)), ("d_kh0", (64, S)),
                        ("d_grow", (1, S)), ("d_gacol", (128, TB)),
                        ("d_cscb", (64, S)), ("d_qtil0", (64, S)),
                         
                        ("d_x2_0", (128, S)), ("d_r2b", (128, S)),
                        ("d_ktok0", (128, TB * 64)), ("d_vtok0", (128, TB * 65))]:
            dt_in[nm] = nc.dram_tensor(nm, list(shp),
                                       BF16 if "tok" in nm else F32,
                                       kind="ExternalOutput").ap()

    with tile.TileContext(nc) as tc:
        _emit(nc, tc, dt_in, yT)

    nc.compile()
    return nc


def _emit(nc, tc, I, yT):
    P_ctx = tc.tile_pool(name="persist", bufs=1)
    dram_ctx = tc.tile_pool(name="dram", bufs=1, space="DRAM")
    with P_ctx as P, dram_ctx as dramp, nc.allow_low_precision('deliberate f32r/bf16 design'):
        def load(pool, name, shape=None, dt=F32, src=None, tag=None):
            src = src if src is not None else I[name]
            shape = list(shape) if shape is not None else list(src.shape)
            t = pool.tile(shape, dt, tag=(tag or name), name=(tag or name))
            nc.sync.dma_start(t[:], src)
            return t

        # ---- small persistent consts ----
        g2_sb = load(P, "g2", dt=F32R)
        mp_sb = load(P, "mp", dt=BF16)
        aa_sb = load(P, "aa")
        bb_sb = load(P, "bb")
        mdiag_sb = load(P, "mdiag", dt=F32R)
        mfar_sb = load(P, "mfar", dt=F32R)
        mones_sb = load(P, "mones", dt=F32R)
        amask_sb = load(P, "amask4", dt=BF16)
        id64_sb = load(P, "ident64", dt=F32R)
        ones65_sb = load(P, "ones65", dt=F32R)
        ones128c_sb = load(P, "ones128c", dt=F32R)
        ones_r_sb = load(P, "ones_r", dt=F32R)
        mg_sb = load(P, "memgate64")
        eps_sb = load(P, "eps11")
        invb64_sb = load(P, "invb64")

        # memory gate scalars, broadcast over 64 partitions
        g_b = P.tile([64, 1], F32, tag="g_b")
        gm1_b = P.tile([64, 1], F32, tag="gm1_b")
        nc.scalar.activation(g_b[:], mg_sb[:], AF.Sigmoid)
        nc.scalar.activation(gm1_b[:], mg_sb[:], AF.Sigmoid, scale=-1.0)

        # mid-lived activation tiles (through attention)
        mid_ctx = tc.tile_pool(name="mid", bufs=1)
        with mid_ctx as M:
            vT = [M.tile([64, S], F32R, tag=f"vT{h}", name=f"vT{h}") for h in range(HPC)]
            qh_t = [M.tile([64, S], F32R, tag=f"qh{h}", name=f"qh{h}") for h in range(HPC)]
            kh_t = [M.tile([64, S], F32R, tag=f"kh{h}", name=f"kh{h}") for h in range(HPC)]
            ga_col = M.tile([128, TB], F32R, tag="ga_col")
            csc_b = M.tile([64, S], F32, tag="csc_b")

            # ============ phase A: qkv + gamma-hidden matmuls ============
            with tc.tile_pool(name="qkvout", bufs=1) as QO:
                qraw = [QO.tile([64, S], F32, tag=f"qraw{h}", name=f"qraw{h}")
                        for h in range(HPC)]
                kraw = [QO.tile([64, S], F32, tag=f"kraw{h}", name=f"kraw{h}")
                        for h in range(HPC)]
                guT = QO.tile([64, S], F32, tag="guT")
                with tc.tile_pool(name="xp", bufs=2) as xp, \
                     tc.tile_pool(name="wqgp", bufs=1) as wp, \
                     tc.tile_pool(name="ps_qkv", bufs=4, space="PSUM") as pq, \
                     tc.tile_pool(name="ps_qx", bufs=2, space="PSUM") as pq2:
                    wqg_sb = [load(wp, "wqg", (128, 448), dt=F32R,
                                   src=I["wqg"][k * 128:(k + 1) * 128, :],
                                   tag=f"wqg{k}") for k in range(KT)]
                    m_outs = [(qraw[0], 0), (qraw[1], 64), (kraw[0], 128),
                              (kraw[1], 192), (vT[0], 256), (vT[1], 320),
                              (guT, 384)]
                    for c in range(NCH):
                        csl = slice(c * 512, (c + 1) * 512)
                        x_sb = []
                        for k in range(KT):
                            t = xp.tile([128, 512], F32R, tag=f"x{k}", name=f"x{k}_{c}")
                            nc.sync.dma_start(
                                t[:], I["xT"][k * 128:(k + 1) * 128, csl])
                            x_sb.append(t)
                        # r1 = rsqrt(mean_d x^2 + eps) for this chunk
                        ps_x = pq2.tile([1, 512], F32, tag="ps_x")
                        for k in range(KT):
                            sqx = xp.tile([128, 512], F32R, tag="sqx")
                            nc.scalar.activation(sqx[:], x_sb[k].bitcast(F32)[:],
                                                 AF.Square)
                            nc.tensor.matmul(ps_x[:], ones128c_sb[:], sqx[:],
                                             start=(k == 0), stop=(k == KT - 1))
                        m1x = xp.tile([1, 512], F32, tag="m1x")
                        nc.scalar.activation(m1x[:], ps_x[:], AF.Identity,
                                             bias=eps_sb[:])
                        rc1 = xp.tile([1, 512], F32, tag="rc1")
                        nc.vector.reciprocal(rc1[:], m1x[:])
                        r1row = xp.tile([1, 512], F32R, tag="r1row")
                        nc.scalar.activation(r1row[:], rc1[:], AF.Sqrt)
                        pb1 = pq2.tile([64, 512], F32, tag="pb1")
                        nc.tensor.matmul(pb1[:], ones_r_sb[:, 0:64], r1row[:],
                                         start=True, stop=True)
                        r1b = xp.tile([64, 512], F32, tag="r1b")
                        nc.any.tensor_copy(r1b[:], pb1[:])
                        for (dst, moff) in m_outs:
                            ps = pq.tile([64, 512], F32, tag="ps")
                            for k in range(KT):
                                nc.tensor.matmul(
                                    ps[:], _f32r(wqg_sb[k][:, moff:moff + 64]),
                                    _f32r(x_sb[k][:]),
                                    start=(k == 0), stop=(k == KT - 1))
                            nc.vector.tensor_mul(dst[:, csl], ps[:], r1b[:])

                if DEBUG:
                    nc.sync.dma_start(I["d_qraw0"], qraw[0].bitcast(F32)[:])
                # ============ gamma gate rows ============
                with tc.tile_pool(name="gp", bufs=2) as gp, \
                     tc.tile_pool(name="ps_g", bufs=1, space="PSUM") as pg:
                    gsil = gp.tile([64, S], F32R, tag="gsil")
                    nc.scalar.activation(gsil[:], guT[:], AF.Silu)
                    grow = gp.tile([1, S], F32, tag="grow")
                    for c in range(NCH):
                        csl = slice(c * 512, (c + 1) * 512)
                        psg = pg.tile([1, 512], F32, tag="psg")
                        nc.tensor.matmul(psg[:], _f32r(g2_sb[:]),
                                         _f32r(gsil[:, csl]), start=True, stop=True)
                        nc.scalar.activation(grow[:, csl], psg[:], AF.Sigmoid)
                    if DEBUG:
                        nc.sync.dma_start(I["d_grow"], grow[:])
                    ga_row = gp.tile([1, S], F32R, tag="ga_row")
                    nc.vector.tensor_mul(ga_row[:], grow[:], aa_sb[:])
                    ga_dram = dramp.tile([1, S], F32R, tag="ga_dram")
                    nc.sync.dma_start(ga_dram[:], ga_row[:])
                    with nc.allow_non_contiguous_dma("one-time gamma transpose"):
                        nc.sync.dma_start(
                            ga_col[:], ga_dram.rearrange("a (j p) -> (a p) j", p=128))

                    # norm row: csc[t] = lam^t / (1 + lam^t * sum_i ga_i mask)
                    with tc.tile_pool(name="ps_bc", bufs=2, space="PSUM") as pbc:
                        for cc in range(NCH):
                            csl = slice(cc * 512, (cc + 1) * 512)
                            ps_ns = pg.tile([1, 512], F32, tag="ps_ns")
                            for tt in range(4):
                                tb = cc * 4 + tt
                                ibs = list(range(max(0, tb - 4), tb + 1))
                                for j, ib in enumerate(ibs):
                                    msk = mdiag_sb if ib == tb else (
                                        mfar_sb if ib == tb - 4 else mones_sb)
                                    nc.tensor.matmul(
                                        ps_ns[:, tt * 128:(tt + 1) * 128],
                                        ga_col[:, ib:ib + 1], msk[:],
                                        start=(j == 0), stop=(j == len(ibs) - 1))
                            nrm = gp.tile([1, 512], F32, tag="nrm")
                            nc.vector.tensor_mul(nrm[:], ps_ns[:], bb_sb[:, csl])
                            nc.vector.tensor_scalar_add(nrm[:], nrm[:], 1.0)
                            rcpn = gp.tile([1, 512], F32, tag="rcpn")
                            nc.vector.reciprocal(rcpn[:], nrm[:])
                            csc_row = gp.tile([1, 512], F32R, tag="csc_row")
                            nc.vector.tensor_mul(csc_row[:], rcpn[:], bb_sb[:, csl])
                            pb = pbc.tile([64, 512], F32, tag="pb")
                            nc.tensor.matmul(pb[:], _f32r(ones_r_sb[:, 0:64]),
                                             _f32r(csc_row[:]), start=True, stop=True)
                            nc.any.tensor_copy(csc_b[:, csl], pb[:])

                # ============ phase B: qk rms-norm + rope (both heads) ============
                with tc.tile_pool(name="ropes", bufs=1) as RT, \
                     tc.tile_pool(name="ropew", bufs=1) as rp, \
                     tc.tile_pool(name="ps_r", bufs=2, space="PSUM") as pr:
                    cosq_sb = load(RT, "cosq"); sinq_sb = load(RT, "sinq")
                    cosk_sb = load(RT, "cosk"); sink_sb = load(RT, "sink")
                    for h in range(HPC):
                        for nm, src, cs, sn, dst in (
                                ("q", qraw[h][:], cosq_sb, sinq_sb, qh_t[h]),
                                ("k", kraw[h][:], cosk_sb, sink_sb, kh_t[h])):
                            # rms scale row: rsqrt(mean + eps)
                            sq = rp.tile([65, S], F32R, tag="sq")
                            nc.vector.tensor_mul(sq[0:64, :], src, src)
                            nc.gpsimd.memset(sq.bitcast(F32)[64:65, :], 6.4e-5)
                            rro = rp.tile([1, S], F32R, tag="rro")
                            for c in range(NCH):
                                csl = slice(c * 512, (c + 1) * 512)
                                ps = pr.tile([1, 512], F32, tag="ps")
                                nc.tensor.matmul(ps[:], _f32r(ones65_sb[:]),
                                                 _f32r(sq[:, csl]), start=True, stop=True)
                                rc = rp.tile([1, 512], F32, tag="rc")
                                nc.vector.reciprocal(rc[:], ps[:])
                                nc.scalar.activation(rro[:, csl], rc[:], AF.Sqrt)
                            # rope
                            sh = rp.tile([64, S], F32, tag="sh")
                            nc.sync.dma_start(sh[0:32, :], src[32:64, :])
                            nc.sync.dma_start(sh[32:64, :], src[0:32, :])
                            t1 = rp.tile([64, S], F32, tag="t1")
                            nc.vector.tensor_mul(t1[:], src, cs[:])
                            nc.gpsimd.tensor_mul(sh[:], sh[:], sn[:])
                            nc.vector.tensor_add(t1[:], t1[:], sh[:])
                            for c in range(NCH):
                                csl = slice(c * 512, (c + 1) * 512)
                                pb = pr.tile([64, 512], F32, tag="pb")
                                nc.tensor.matmul(pb[:], _f32r(ones_r_sb[:, 0:64]),
                                                 _f32r(rro[:, csl]), start=True, stop=True)
                                nc.vector.tensor_mul(dst[:, csl], t1[:, csl], pb[:])

            if DEBUG:
                nc.sync.dma_start(I["d_gacol"], ga_col.bitcast(F32)[:])
                nc.sync.dma_start(I["d_cscb"], csc_b[:])
                nc.sync.dma_start(I["d_kh0"], kh_t[0].bitcast(F32)[:])
            # ============ phase C: per-head omega + attention ============
            aop_cm = tc.tile_pool(name="aop", bufs=1)
            aop = aop_cm.__enter__()
            ao_t = [aop.tile([64, S], BF16, tag=f"ao{h}", name=f"ao{h}")
                    for h in range(HPC)]
            for h in range(HPC):
                qh, kh, vraw = qh_t[h], kh_t[h], vT[h]
                with tc.tile_pool(name=f"head{h}", bufs=1) as hp:
                    qhb = hp.tile([64, S], BF16, tag="qhb")
                    nc.vector.tensor_copy(qhb[:], qh.bitcast(F32)[:])
                    qomb = hp.tile([64, S], BF16, tag="qomb")
                    nc.vector.tensor_mul(qomb[:], qh.bitcast(F32)[:], invb64_sb[:])
                    khb = hp.tile([64, S], BF16, tag="khb")
                    nc.vector.tensor_copy(khb[:], kh.bitcast(F32)[:])

                    ktok = hp.tile([128, TB * 64], BF16, tag="ktok")
                    vtok = hp.tile([128, TB * 65], BF16, tag="vtok")
                    nc.sync.dma_start(vtok[:], I["vones"])
                    with tc.tile_pool(name=f"tp{h}", bufs=4, space="PSUM") as tp:
                        for j in range(TB):
                            pt = tp.tile([128, 64], F32R, tag="pt")
                            nc.tensor.transpose(pt[:], kh[:, j * 128:(j + 1) * 128],
                                                id64_sb[:])
                            nc.any.tensor_copy(ktok[:, j * 64:(j + 1) * 64], pt.bitcast(F32)[:])
                            pt2 = tp.tile([128, 64], F32R, tag="pt2")
                            nc.tensor.transpose(pt2[:], vraw[:, j * 128:(j + 1) * 128],
                                                id64_sb[:])
                            nc.any.tensor_copy(vtok[:, j * 65:j * 65 + 64], pt2.bitcast(F32)[:])

                    # ---- omega -> qtil ----
                    qtil = hp.tile([64, S], F32R, tag="qtil")
                    with tc.tile_pool(name=f"om{h}", bufs=3) as om, \
                         tc.tile_pool(name=f"ps_s{h}", bufs=2, space="PSUM") as pss, \
                         tc.tile_pool(name=f"ps_c{h}", bufs=2, space="PSUM") as psc:
                        for ch in range(NCH):
                            ps_c4 = psc.tile([64, 512], F32, tag="ps_c4")
                            for tt in range(4):
                                tb = ch * 4 + tt
                                ibs = list(range(max(0, tb - 4), tb + 1))
                                ps_s = pss.tile([128, 640], F32, tag="ps_s")
                                ws = om.tile([128, 640], BF16, tag="ws")
                                for j, ib in enumerate(ibs):
                                    jsl = (slice(None), slice(j * 128, (j + 1) * 128))
                                    nc.tensor.matmul(ps_s[jsl],
                                                     khb[:, ib * 128:(ib + 1) * 128],
                                                     qhb[:, tb * 128:(tb + 1) * 128],
                                                     start=True, stop=True)
                                    gcol = ga_col.bitcast(F32)[:, ib:ib + 1]
                                    if ib == tb:
                                        nc.vector.scalar_tensor_tensor(
                                            ws[jsl], ps_s[jsl], gcol, mdiag_sb.bitcast(F32)[:],
                                            op0=ALU.mult, op1=ALU.mult)
                                    elif ib == tb - 4:
                                        nc.vector.scalar_tensor_tensor(
                                            ws[jsl], ps_s[jsl], gcol, mfar_sb.bitcast(F32)[:],
                                            op0=ALU.mult, op1=ALU.mult)
                                    else:
                                        nc.vector.tensor_scalar_mul(
                                            ws[jsl], ps_s[jsl], gcol)
                                osl = (slice(None), slice(tt * 128, (tt + 1) * 128))
                                nc.tensor.matmul(ps_c4[osl],
                                                 mp_sb[:, h * 64:(h + 1) * 64],
                                                 qomb[:, tb * 128:(tb + 1) * 128],
                                                 start=True, stop=False)
                                for j, ib in enumerate(ibs):
                                    nc.tensor.matmul(ps_c4[osl],
                                                     ktok[:, ib * 64:(ib + 1) * 64],
                                                     ws[:, j * 128:(j + 1) * 128],
                                                     start=False,
                                                     stop=(j == len(ibs) - 1))
                            csl = slice(ch * 512, (ch + 1) * 512)
                            tq = om.tile([64, 512], F32, tag="tq")
                            nc.vector.scalar_tensor_tensor(
                                tq[:], ps_c4[:], g_b[:], csc_b[:, csl],
                                op0=ALU.mult, op1=ALU.mult)
                            nc.vector.scalar_tensor_tensor(
                                qtil[:, csl], qh.bitcast(F32)[:, csl], gm1_b[:], tq[:],
                                op0=ALU.mult, op1=ALU.add)

                    if DEBUG and h == 0:
                        nc.sync.dma_start(I["d_qtil0"], qtil.bitcast(F32)[:])
                        nc.sync.dma_start(I["d_ktok0"], ktok[:])
                        nc.sync.dma_start(I["d_vtok0"], vtok[:])
                    # ---- causal attention ----
                    avsp_cm = tc.tile_pool(name=f"avs{h}", bufs=1)
                    avsp = avsp_cm.__enter__()
                    with tc.tile_pool(name=f"at{h}", bufs=3) as at, \
                         tc.tile_pool(name=f"ps_a{h}", bufs=2, space="PSUM") as psa, \
                         tc.tile_pool(name=f"ps_av{h}", bufs=1, space="PSUM") as psav:
                        av_ps = [psav.tile([65, 512], F32, tag=f"av{c}",
                                           name=f"av{h}_{c}") for c in range(NCH)]
                        avs = [avsp.tile([65, 512], F32, tag=f"avs{c}",
                                         name=f"avs{h}_{c}") for c in range(NCH)]
                        for half in range(2):
                            for ib in range(8 * half + 8):
                                ps_a = psa.tile([128, 1024], F32, tag="ps_a")
                                ab = at.tile([128, 1024], BF16, tag="ab")
                                vs = None
                                for sc in range(2):
                                    c = half * 2 + sc
                                    if (c + 1) * 512 <= ib * 128:
                                        continue
                                    if vs is None:
                                        vs = sc
                                    nc.tensor.matmul(
                                        ps_a[:, sc * 512:(sc + 1) * 512],
                                        _f32r(kh[:, ib * 128:(ib + 1) * 128]),
                                        _f32r(qtil[:, (c * 512):(c + 1) * 512]),
                                        start=True, stop=True)
                                esl = (slice(None), slice(vs * 512, 1024))
                                nc.scalar.activation(ab[esl], ps_a[esl], AF.Exp,
                                                     scale=HD ** -0.5)
                                cd = (ib * 128) // 512
                                r = ib - 4 * cd
                                scd = cd - 2 * half
                                if scd >= 0:
                                    dsl = (slice(None),
                                           slice(scd * 512, (scd + 1) * 512))
                                    nc.vector.tensor_mul(
                                        ab[dsl], ab[dsl],
                                        amask_sb[:, r * 512:(r + 1) * 512])
                                for sc in range(2):
                                    c = half * 2 + sc
                                    if (c + 1) * 512 <= ib * 128:
                                        continue
                                    nc.tensor.matmul(
                                        av_ps[c][:], vtok[:, ib * 65:(ib + 1) * 65],
                                        ab[:, sc * 512:(sc + 1) * 512],
                                        start=(ib == 0), stop=(ib == 4 * c + 3))
                            for sc in range(2):
                                c = half * 2 + sc
                                nc.any.tensor_copy(avs[c][:], av_ps[c][:])
                    # normalize: aoT = avs[:64] * (1/avs[64]) broadcast
                    with tc.tile_pool(name=f"nz{h}", bufs=2) as nz, \
                         tc.tile_pool(name=f"ps_rc{h}", bufs=2, space="PSUM") as prc:
                        for c in range(NCH):
                            csl = slice(c * 512, (c + 1) * 512)
                            rcp = nz.tile([1, 512], F32R, tag="rcp")
                            nc.vector.reciprocal(rcp[:], avs[c][64:65, :])
                            pb = prc.tile([64, 512], F32, tag="pb")
                            nc.tensor.matmul(pb[:], _f32r(ones_r_sb[:, 0:64]),
                                             _f32r(rcp[:]), start=True, stop=True)
                            nc.vector.tensor_mul(
                                ao_t[h][:, csl], avs[c][0:64, :], pb[:])
                    avsp_cm.__exit__(None, None, None)

            # ====== phase D: AllGather ao (bf16), local o_proj ======
            ao_loc = dramp.tile([128, S], BF16, tag="ao_loc")
            ao_all = dramp.tile([D, S], BF16, tag="ao_all", addr_space="Shared")
            nc.sync.dma_start(ao_loc[0:64, :], ao_t[0][:])
            nc.sync.dma_start(ao_loc[64:128, :], ao_t[1][:])
            nc.gpsimd.collective_compute(
                "AllGather", ALU.bypass, replica_groups=[list(range(NCORES))],
                ins=[ao_loc[:]], outs=[ao_all[:]])
            aop_cm.__exit__(None, None, None)

        # ====== phase E: o_proj from gathered heads, x2 = x + o_proj, rms2 ======
        with tc.tile_pool(name="x2p", bufs=1) as x2p:
            x2_sb = [x2p.tile([128, S], F32R, tag=f"x2_{d}", name=f"x2_{d}")
                     for d in range(KT)]
            r2b = x2p.tile([128, S], F32, tag="r2b")
            with tc.tile_pool(name="sqp", bufs=2) as sqp, \
                 tc.tile_pool(name="aap", bufs=1) as aap, \
                 tc.tile_pool(name="wobp", bufs=1) as wobp, \
                 tc.tile_pool(name="ps_s2", bufs=1, space="PSUM") as ps2p, \
                 tc.tile_pool(name="ps_op", bufs=2, space="PSUM") as psop:
                wob_sb = [load(wobp, "wob", (128, D), dt=BF16,
                               src=I["wob"][k * 128:(k + 1) * 128, :],
                               tag=f"wob{k}") for k in range(KT)]
                aoall_sb = [load(aap, "ao_all", (128, S), dt=BF16,
                                 src=ao_all[k * 128:(k + 1) * 128, :],
                                 tag=f"aoall{k}") for k in range(KT)]
                ps_sq = [ps2p.tile([1, 512], F32, tag=f"ps_sq{c}", name=f"ps_sq{c}")
                         for c in range(NCH)]
                for m in range(KT):
                    tx = sqp.tile([128, S], F32R, tag="tx")
                    nc.sync.dma_start(tx[:], I["xT"][m * 128:(m + 1) * 128, :])
                    for c in range(NCH):
                        csl = slice(c * 512, (c + 1) * 512)
                        ps_o = psop.tile([128, 512], F32, tag="ps_o")
                        for k in range(KT):
                            nc.tensor.matmul(ps_o[:], wob_sb[k][:, m * 128:(m + 1) * 128],
                                             aoall_sb[k][:, csl],
                                             start=(k == 0), stop=(k == KT - 1))
                        nc.vector.tensor_add(x2_sb[m][:, csl], ps_o[:],
                                             tx.bitcast(F32)[:, csl])
                    sq = sqp.tile([128, S], F32R, tag="sq")
                    nc.scalar.activation(sq[:], x2_sb[m].bitcast(F32)[:], AF.Square)
                    for c in range(NCH):
                        csl = slice(c * 512, (c + 1) * 512)
                        nc.tensor.matmul(ps_sq[c][:], ones128c_sb[:],
                                         _f32r(sq[:, csl]),
                                         start=(m == 0), stop=(m == KT - 1))
                r2row = sqp.tile([1, S], F32R, tag="r2row")
                for c in range(NCH):
                    m2 = sqp.tile([1, 512], F32, tag="m2")
                    nc.scalar.activation(m2[:], ps_sq[c][:], AF.Identity, bias=eps_sb[:])
                    rc2 = sqp.tile([1, 512], F32, tag="rc2")
                    nc.vector.reciprocal(rc2[:], m2[:])
                    nc.scalar.activation(r2row[:, c * 512:(c + 1) * 512],
                                         rc2[:], AF.Sqrt)
                with tc.tile_pool(name="ps_b2", bufs=2, space="PSUM") as pb2:
                    for c in range(NCH):
                        csl = slice(c * 512, (c + 1) * 512)
                        pb = pb2.tile([128, 512], F32, tag="pb")
                        nc.tensor.matmul(pb[:], ones_r_sb[:],
                                         r2row[:, csl], start=True, stop=True)
                        nc.any.tensor_copy(r2b[:, csl], pb[:])

            # ============ phase F: FFN (column shard) + residual/8 ============
            with tc.tile_pool(name="ffn", bufs=3) as fp, \
                 tc.tile_pool(name="gpool", bufs=1) as gpl, \
                 tc.tile_pool(name="w123", bufs=1) as wfp, \
                 tc.tile_pool(name="ps_f", bufs=2, space="PSUM") as pf:
                w1_sb = [load(wfp, "w1", (128, FC), dt=F32R,
                              src=I["w1"][k * 128:(k + 1) * 128, :],
                              tag=f"w1{k}") for k in range(KT)]
                w3_sb = [load(wfp, "w3", (128, FC), dt=F32R,
                              src=I["w3"][k * 128:(k + 1) * 128, :],
                              tag=f"w3{k}") for k in range(KT)]
                w2_sb = [load(wfp, "w2b", (F_SIZES[f], D), dt=BF16,
                              src=I["w2b"][F_OFFS[f]:F_OFFS[f] + F_SIZES[f], :],
                              tag=f"w2{f}") for f in range(3)]
                g_sb = [gpl.tile([F_SIZES[f], S], BF16, tag=f"g{f}", name=f"g{f}")
                        for f in range(3)]
                for f in range(3):
                    fo, fs = F_OFFS[f], F_SIZES[f]
                    for c in range(NCH):
                        csl = slice(c * 512, (c + 1) * 512)
                        ps1 = pf.tile([fs, 512], F32, tag="ps1")
                        ps3 = pf.tile([fs, 512], F32, tag="ps3")
                        for k in range(KT):
                            nc.tensor.matmul(ps1[:], _f32r(w1_sb[k][:, fo:fo + fs]),
                                             _f32r(x2_sb[k][:, csl]),
                                             start=(k == 0), stop=(k == KT - 1))
                        for k in range(KT):
                            nc.tensor.matmul(ps3[:], _f32r(w3_sb[k][:, fo:fo + fs]),
                                             _f32r(x2_sb[k][:, csl]),
                                             start=(k == 0), stop=(k == KT - 1))
                        u1 = fp.tile([fs, 512], F32, tag="u1")
                        nc.vector.tensor_mul(u1[:], ps1[:], r2b[0:fs, csl])
                        su = fp.tile([fs, 512], F32, tag="su")
                        nc.scalar.activation(su[:], u1[:], AF.Silu)
                        u3 = fp.tile([fs, 512], F32, tag="u3")
                        nc.vector.tensor_mul(u3[:], ps3[:], r2b[0:fs, csl])
                        nc.vector.tensor_mul(g_sb[f][:, csl], su[:], u3[:])
                for m in range(KT):
                    for c in range(NCH):
                        csl = slice(c * 512, (c + 1) * 512)
                        psy = pf.tile([128, 512], F32, tag="psy")
                        for f in range(3):
                            nc.tensor.matmul(psy[:], w2_sb[f][:, m * 128:(m + 1) * 128],
                                             g_sb[f][:, csl],
                                             start=(f == 0), stop=(f == 2))
                        ysb = fp.tile([128, 512], F32, tag="ysb")
                        eng = nc.vector
                        eng.scalar_tensor_tensor(ysb[:], x2_sb[m].bitcast(F32)[:, csl],
                                                 1.0 / NCORES, psy[:],
                                                 op0=ALU.mult, op1=ALU.add)
                        nc.sync.dma_start(yT[m * 128:(m + 1) * 128, csl], ysb[:])


_NC_CACHE = None


def _get_program():
    global _NC_CACHE
    if _NC_CACHE is None:
        _NC_CACHE = build_program()
    return _NC_CACHE


def _host_prepare(inputs):
    x = np.asarray(inputs["x"], np.float32)
    n1 = np.asarray(inputs["norm1_w"], np.float32)
    n2 = np.asarray(inputs["norm2_w"], np.float32)
    w_qkv = np.asarray(inputs["w_qkv"], np.float32)
    qn = np.asarray(inputs["q_norm_w"], np.float32)
    kn = np.asarray(inputs["k_norm_w"], np.float32)
    gw1 = np.asarray(inputs["gamma_w1"], np.float32)
    gw2 = np.asarray(inputs["gamma_w2"], np.float32)
    mper = np.asarray(inputs["m_persist"], np.float32)
    mg = np.asarray(inputs["memory_gate"], np.float32)
    w_o = np.asarray(inputs["w_o"], np.float32)
    fw1 = np.asarray(inputs["ffn_w1"], np.float32)
    fw3 = np.asarray(inputs["ffn_w3"], np.float32)
    fw2 = np.asarray(inputs["ffn_w2"], np.float32)

    xT = np.ascontiguousarray(x[0].T)

    t = np.arange(S, dtype=np.float32)
    inv = (1.0 / (10000.0 ** (np.arange(0, HD, 2, dtype=np.float32) / HD)))
    fr = np.outer(inv, t)                       # [32, S]
    c32, s32 = np.cos(fr), np.sin(fr)
    cos64 = np.concatenate([c32, c32], 0)       # [64, S]
    sin64 = np.concatenate([s32, s32], 0)
    sign = np.where(np.arange(HD) < 32, -1.0, 1.0).astype(np.float32)[:, None]

    def rope_tabs(w):
        cosw = cos64 * w[:, None]
        wsh = np.concatenate([w[32:], w[:32]])
        sinw = sin64 * sign * wsh[:, None]
        return (np.ascontiguousarray(cosw, dtype=np.float32),
                np.ascontiguousarray(sinw, dtype=np.float32))

    cosq, sinq = rope_tabs(qn)
    cosk, sink = rope_tabs(kn)

    i_idx = np.arange(S, dtype=np.float64)
    aa = (LAM ** (-i_idx)).astype(np.float32)[None, :]
    bb = (LAM ** i_idx).astype(np.float32)[None, :]
    invb64 = np.broadcast_to(aa, (64, S)).astype(np.float32)

    ii = np.arange(128)[:, None]
    u = np.arange(512)[None, :]
    mdiag = (np.arange(128)[None, :] >= ii).astype(np.float32)
    mfar = (np.arange(128)[None, :] < ii).astype(np.float32)
    mones = np.ones((128, 128), np.float32)
    amask4 = np.concatenate(
        [(u >= r * 128 + ii).astype(np.float32) for r in range(4)], axis=1
    ).astype(ml_dtypes.bfloat16)

    fw1p = np.zeros((D, NCORES * FC), np.float32)
    fw1p[:, :FFN_H] = fw1 * n2[:, None]
    fw3p = np.zeros((D, NCORES * FC), np.float32)
    fw3p[:, :FFN_H] = fw3 * n2[:, None]
    fw2p = np.zeros((NCORES * FC, D), np.float32)
    fw2p[:FFN_H, :] = fw2

    wob = np.ascontiguousarray(w_o.astype(ml_dtypes.bfloat16))
    vones = np.zeros((128, TB * 65), ml_dtypes.bfloat16)
    for j in range(TB):
        vones[:, j * 65 + 64] = 1.0

    in_maps = []
    for c in range(NCORES):
        h0, h1 = HPC * c, HPC * c + 1
        cols = []
        for base in (0, D, 2 * D):
            for hh in (h0, h1):
                cols.append(w_qkv[:, base + hh * HD: base + (hh + 1) * HD])
        cols.append(gw1)
        wqg = np.concatenate(cols, axis=1) * n1[:, None]
        mp = np.concatenate([mper[h0].T, mper[h1].T], axis=1).astype(ml_dtypes.bfloat16)
        in_maps.append({
            "xT": xT, "wqg": np.ascontiguousarray(wqg),
            "g2": gw2,
            "cosq": cosq, "sinq": sinq, "cosk": cosk, "sink": sink,
            "mp": np.ascontiguousarray(mp),
            "wob": wob,
            "w1": np.ascontiguousarray(fw1p[:, c * FC:(c + 1) * FC]),
            "w3": np.ascontiguousarray(fw3p[:, c * FC:(c + 1) * FC]),
            "w2b": np.ascontiguousarray(
                fw2p[c * FC:(c + 1) * FC, :].astype(ml_dtypes.bfloat16)),
            "aa": aa, "invb64": invb64, "bb": bb,
            "mdiag": mdiag, "mfar": mfar, "mones": mones,
            "amask4": np.ascontiguousarray(amask4),
            "ident64": np.eye(HD, dtype=np.float32),
            "ones65": np.full((65, 1), 1.0 / HD, np.float32),
            "ones128c": np.full((128, 1), 1.0 / D, np.float32),
            "ones_r": np.ones((1, 128), np.float32),
            "memgate64": np.full((64, 1), float(mg.reshape(-1)[0]), np.float32),
            "eps11": np.full((1, 1), 1e-6, np.float32),
            "vones": vones,
        })
    return in_maps


def kernel(**inputs):
    nc = _get_program()
    in_maps = _host_prepare(inputs)
    res = run_bass_kernel_spmd(nc, in_maps, list(range(NCORES)))
    acc = None
    for c in range(NCORES):
        yt = np.asarray(res.results[c]["yT"], np.float32)
        acc = yt.copy() if acc is None else acc + yt
    return np.ascontiguousarray(acc.T)[None].astype(np.float32)
